# revision 28
# baseline (speedup 1.0000x reference)
"""IoU loss kernel for Trainium2, data-parallel over 8 NeuronCores.

Math (per box, columns = x-center, y-center, half-size s):
    w = relu(min(x+s, x'+s') - max(x-s, x'-s'))
      = relu((s+s') - max(|x-x'|, |s-s'|))          # S - max identity
    h likewise with y.
    overlap = w*h
    union   = 4s^2 + 4s'^2 - overlap = 2(S^2 + D^2) - overlap,
              S = s+s', D = s-s'
    iou     = overlap / (union + 1e-7)
    loss    = -sum(log(iou + 1e-7));  iou_sum = sum(iou)

Engine-assignment rationale. The SBUF fabric saturates when several
engines stream wide operands concurrently, and DVE instructions (the
in-order bottleneck queue) inflate 1.5-2.3x under that load; PE
matmuls never inflate (private weight/moving-data path + PSUM
output). So everything that CAN be a matmul IS one:

  PE   : SP = I*s1 + I*s2 (fp32 identity matmuls straight off the
         interleaved raw fp32 columns -> PSUM, replacing a DVE strided
         add that measured 3.5us/seg), ue = eps + 2S^2 + 2D^2 - ov
         accumulated in PSUM (eps enters via a bf16 rank-full matmul
         epsw.T @ ones2 = 128 * eps/128), and the ones-weight matmuls
         accumulating per-partition iou sums across all segments. PE
         drains also absorb the raw-slot recycle waits.
  DVE  (~9.8us/KTile): m3 = rawO - rawT, ONE contiguous fp32 subtract
         covering dx, dy, D (four strided reads would re-fetch the
         same 32B beats four times), mw/mh = max(a., aD) (fp16 2x),
         wr/hr = SP - m. (PSUM-operand tensor_tensor), and
         r = 1/(u+eps) via reciprocal_approx_fast off PSUM.
  ACT  (~8.3us/KTile): |dx|, |dy|, |D| (Abs on m3's strided columns -
         ACT is stride-blind), 2D^2/2S^2 (Square with scale=sqrt2; qS
         reads the SP PSUM banks), relu(wr)/relu(hr), Ln(iou+eps)
         accum -> loss partial. All funcs live in the `natural_log`
         table set -> ONE table load total.
  Pool (~5.2us/KTile): ov = rw*rh, iou = ov*r (fp16*fp32), the two
         Q7-ucode tensor_tensor slots.

PSUM discipline: the SP and ue banks are single-buffered and shared
across segments; every reader of generation s is ISSUED before the
generation s+1 matmuls so the tile framework sees a WAR (serialize
correctly) instead of a wrong-generation RAW. PE has slack to absorb
the resulting waits.

Six-iteration software pipeline; trailing/leading segments shrink
(512) to cut fill and drain. Host: final [128, NSEG] x 8 cores loss
partials + [1,1024] iou partials, summed in float64.
"""

import numpy as np

import concourse.bass as bass
import concourse.mybir as mybir
from concourse import tile
from concourse.bass_utils import run_bass_kernel_spmd

N = 8388608
NCORES = 8
NS = N // NCORES  # 1048576 boxes per core
P = 128
W = 1024          # boxes per partition per full tile
T = NS // (P * W)  # 8 full-tile units per core
EPS = 1e-7
RT2 = 1.4142135623730951

F32 = mybir.dt.float32
F16 = mybir.dt.float16
Op = mybir.AluOpType
Act = mybir.ActivationFunctionType


def _build(T_: int = T, W_: int = W, compile_passes: bool = True) -> bass.Bass:
    from concourse import bacc
    from concourse.tile_rust import add_dep_helper

    # small leading segment cuts the pipeline-fill stall (DVE waits on
    # the first DMA); small trailing segment cuts the drain-out tail.
    segs = [W_ // 2] + [W_] * (T_ - 1) + [W_ // 2]
    assert sum(segs) == T_ * W_
    NSEG = len(segs)
    SMALLW = W_ // 2
    H = W_ // 2  # psum bank width (512 fp32 cols)
    BIG_LO, BIG_HI = 1, T_ - 1  # segs with width W_ (inclusive range)

    ns = P * W_ * T_
    nc = bacc.Bacc()
    outs_d = nc.dram_tensor("outputs", [ns, 3], F32, kind="ExternalInput")
    tars_d = nc.dram_tensor("targets", [ns, 3], F32, kind="ExternalInput")
    acc_d = nc.dram_tensor("acc", [P, NSEG], F32, kind="ExternalOutput")
    iouv_d = nc.dram_tensor("iouv", [1, 2 * H], F32, kind="ExternalOutput")

    offs = [0]
    for w in segs:
        offs.append(offs[-1] + w)

    def seg_view(dram, s):
        b0 = P * offs[s]
        return dram[b0 : b0 + P * segs[s], :].rearrange(
            "(p w) c -> p (w c)", p=P, w=segs[s]
        )

    RAWBUFS = 3

    with tile.TileContext(nc) as tc:
        with (tc.tile_pool(name="main", bufs=2) as pool,
              tc.tile_pool(name="psum", bufs=1,
                           space=bass.MemorySpace.PSUM) as psum):
            accs = pool.tile([P, NSEG], F32, tag="accs", bufs=1)
            nc.vector.memset(accs[:, :], 0.0)
            eps_t = pool.tile([P, 1], F32, tag="eps", bufs=1)
            nc.vector.memset(eps_t[:, :], EPS)
            ones = pool.tile([P, 1], F16, tag="ones", bufs=1)
            nc.vector.memset(ones[:, :], 1.0)

            # identity / -identity weights: iota(col - partition) is exact
            # in fp16 (range +-127), is_equal against 0 marks the diagonal.
            colmp = pool.tile([P, P], F16, tag="colmp", bufs=1)
            nc.gpsimd.iota(colmp[:, :], [[1, P]], channel_multiplier=-1,
                           allow_small_or_imprecise_dtypes=True)
            ident = pool.tile([P, P], F16, tag="ident", bufs=1)
            nc.vector.tensor_scalar(ident[:, :], colmp[:, :], 0.0, None,
                                    Op.is_equal)
            nident = pool.tile([P, P], F16, tag="nident", bufs=1)
            nc.vector.tensor_scalar(nident[:, :], colmp[:, :], 0.0, -1.0,
                                    Op.is_equal, Op.mult)
            identF = pool.tile([P, P], F32, tag="identF", bufs=1)
            nc.vector.tensor_copy(identF[:, :], ident[:, :])

            # eps enters the union inside PSUM: a bf16 rank-full matmul
            # epsw.T @ ones2 adds 128 * (eps/128) = eps to every element.
            BF16 = mybir.dt.bfloat16
            epsw = pool.tile([P, P], BF16, tag="epsw", bufs=1)
            nc.vector.memset(epsw[:, :], EPS / P)
            ones2 = pool.tile([P, H], BF16, tag="ones2", bufs=1)
            nc.vector.memset(ones2[:, :], 1.0)

            # PSUM: iou partition-sum accumulators (whole kernel) plus the
            # shared single-buffered SP (s1+s2) and ue banks; see module
            # docstring for the program-order WAR discipline.
            psA1 = psum.tile([1, H], F32, tag="psA1", bufs=1)
            psA2 = psum.tile([1, H], F32, tag="psA2", bufs=1)
            ueP1 = psum.tile([P, H], F32, tag="ueP1", bufs=1)
            ueP2 = psum.tile([P, H], F32, tag="ueP2", bufs=1)
            # SP banks alternate by segment parity: spmm(s+1) is issued
            # (iteration s+1) before the segment-s extents are read
            # (iteration s+2), so a single shared pair would hand mids the
            # wrong generation. Two pairs give 2-segment WAR spacing.
            SPa1 = psum.tile([P, H], F32, tag="SPa1", bufs=1)
            SPa2 = psum.tile([P, H], F32, tag="SPa2", bufs=1)
            SPb1 = psum.tile([P, H], F32, tag="SPb1", bufs=1)
            SPb2 = psum.tile([P, H], F32, tag="SPb2", bufs=1)

            lastrd: list = []
            spmm_h: list = []
            dmaO_h: list = []
            dmaT_h: list = []
            big_idx: list = []
            C: list = []

            def front(t):
                w = segs[t]
                if w <= SMALLW:
                    rawO = pool.tile([P, 3 * SMALLW], F32, tag="rawOs", bufs=3)
                    rawT = pool.tile([P, 3 * SMALLW], F32, tag="rawTs", bufs=3)
                    recycle = None
                else:
                    rawO = pool.tile([P, 3 * W_], F32, tag="rawO", bufs=RAWBUFS)
                    rawT = pool.tile([P, 3 * W_], F32, tag="rawT", bufs=RAWBUFS)
                    nbig = len(big_idx)
                    recycle = big_idx[nbig - RAWBUFS] if nbig >= RAWBUFS else None
                    big_idx.append(t)
                deps = []
                if recycle is not None:
                    dr = nc.tensor.drain(fusable=False)
                    add_dep_helper(dr.ins, lastrd[recycle].ins, sync=True,
                                   reason="absorb DVE WAR tick")
                    for mm in spmm_h[recycle]:
                        add_dep_helper(dr.ins, mm.ins, sync=True,
                                       reason="absorb PE raw-read WAR")
                    add_dep_helper(dr.ins, dmaO_h[recycle].ins, sync=True,
                                   reason="absorb old rawO DMA lane")
                    add_dep_helper(dr.ins, dmaT_h[recycle].ins, sync=True,
                                   reason="absorb old rawT DMA lane")
                    deps = [dr]
                dmaO = nc.sync.dma_start(out=rawO[:, : 3 * w], in_=seg_view(outs_d, t))
                dmaT = nc.sync.dma_start(out=rawT[:, : 3 * w], in_=seg_view(tars_d, t))
                for d in deps:
                    add_dep_helper(dmaO.ins, d.ins, sync=True,
                                   reason="slot guarded by PE drain")
                    add_dep_helper(dmaT.ins, d.ins, sync=True,
                                   reason="slot guarded by PE drain")
                dmaO_h.append(dmaO)
                dmaT_h.append(dmaT)

                # m3 depends on BOTH input DMAs but has one sync-wait
                # slot; absorb rawT's semaphore with a tiny same-queue copy.
                dummy = pool.tile([P, 1], F32, tag="dummy")
                nc.vector.tensor_copy(dummy[:, :], rawT[:, 0:1])

                c = {"t": t, "w": w, "rawO": rawO, "rawT": rawT}
                c["m3"] = pool.tile([P, 3 * W_], F16, tag="m3", bufs=2,
                                    name=f"m3_{t}")
                for nm, nb in (("adx", 2), ("ady", 2), ("aD", 2),
                               ("mw", 2), ("mh", 2), ("wr", 2), ("hr", 2),
                               ("rw", 2), ("rh", 2), ("qS", 3), ("qD", 3),
                               ("q12", 2), ("ov", 2), ("iou", 2), ("sc", 1)):
                    c[nm] = pool.tile([P, W_], F16, tag=nm, bufs=nb,
                                      name=f"{nm}_{t}")
                c["r"] = pool.tile([P, W_], F32, tag="r", name=f"r_{t}")
                c["ue_banks"] = ([(ueP1, 0)] if w <= H
                                 else [(ueP1, 0), (ueP2, H)])
                b1, b2 = (SPa1, SPa2) if t % 2 == 0 else (SPb1, SPb2)
                c["sp_banks"] = [(b1, 0)] if w <= H else [(b1, 0), (b2, H)]

                # the whole AoS de-interleave collapses into ONE contiguous
                # fp32 subtract (full 32B-beat utilization); the strided
                # column extraction happens on ACT, which is stride-blind.
                lastrd.append(
                    nc.vector.tensor_tensor(c["m3"][:, : 3 * w],
                                            rawO[:, : 3 * w], rawT[:, : 3 * w],
                                            Op.subtract))
                C.append(c)

            def spmm(c):  # PE: SP = s1 + s2 off the raw strided fp32 cols
                w = c["w"]
                o1 = c["rawO"][:, : 3 * w].rearrange("p (w c) -> p w c", c=3)
                t1 = c["rawT"][:, : 3 * w].rearrange("p (w c) -> p w c", c=3)
                s1, s2 = o1[:, :, 2], t1[:, :, 2]
                hs = []
                for bank, o in c["sp_banks"]:
                    hs.append(nc.tensor.matmul(bank[:, :], identF[:, :],
                                               s1[:, o : o + H],
                                               start=True, stop=False))
                    hs.append(nc.tensor.matmul(bank[:, :], identF[:, :],
                                               s2[:, o : o + H],
                                               start=False, stop=True))
                spmm_h.append(hs)

            def absd(c):  # ACT: |dx|, |dy|, |D| off m3's interleaved
                # columns, 2D^2 likewise, 2S^2 off the SP PSUM banks
                w = c["w"]
                d3 = c["m3"][:, : 3 * w].rearrange("p (w c) -> p w c", c=3)
                dx, dy, D = d3[:, :, 0], d3[:, :, 1], d3[:, :, 2]
                nc.scalar.activation(c["adx"][:, :w], dx, Act.Abs)
                nc.scalar.activation(c["ady"][:, :w], dy, Act.Abs)
                nc.scalar.activation(c["aD"][:, :w], D, Act.Abs)
                nc.scalar.activation(c["qD"][:, :w], D, Act.Square, scale=RT2)
                for bank, o in c["sp_banks"]:
                    nc.scalar.activation(c["qS"][:, o : o + H], bank[:, :],
                                         Act.Square, scale=RT2)

            def mids(c):  # DVE: thresholds (fp16 2x), extents off SP PSUM
                w = c["w"]
                nc.vector.tensor_tensor(c["mw"][:, :w], c["adx"][:, :w],
                                        c["aD"][:, :w], Op.max)
                nc.vector.tensor_tensor(c["mh"][:, :w], c["ady"][:, :w],
                                        c["aD"][:, :w], Op.max)
                for bank, o in c["sp_banks"]:
                    nc.vector.tensor_tensor(c["wr"][:, o : o + H], bank[:, :],
                                            c["mw"][:, o : o + H], Op.subtract)
                    nc.vector.tensor_tensor(c["hr"][:, o : o + H], bank[:, :],
                                            c["mh"][:, o : o + H], Op.subtract)

            def relus(c):  # ACT: clamp both extents
                w = c["w"]
                nc.scalar.activation(c["rw"][:, :w], c["wr"][:, :w], Act.Relu)
                nc.scalar.activation(c["rh"][:, :w], c["hr"][:, :w], Act.Relu)

            def q12st(c):  # Pool: q12 = 2S^2 + 2D^2
                w = c["w"]
                nc.gpsimd.tensor_tensor(c["q12"][:, :w], c["qS"][:, :w],
                                        c["qD"][:, :w], Op.add)

            def ovst(c):  # Pool: ov = rw*rh
                w = c["w"]
                nc.gpsimd.tensor_tensor(c["ov"][:, :w], c["rw"][:, :w],
                                        c["rh"][:, :w], Op.mult)

            def uemm(c):  # PE: ue = eps + q12 - ov in PSUM
                for bank, o in c["ue_banks"]:
                    nc.tensor.matmul(bank[:, :], epsw[:, :], ones2[:, :],
                                     start=True, stop=False)
                    nc.tensor.matmul(bank[:, :], ident[:, :],
                                     c["q12"][:, o : o + H], start=False, stop=False)
                    nc.tensor.matmul(bank[:, :], nident[:, :],
                                     c["ov"][:, o : o + H], start=False, stop=True)

            def recip(c):  # DVE: r = 1/(u+eps), ~18 bits, straight off PSUM
                for bank, o in c["ue_banks"]:
                    nc.vector.reciprocal_approx_fast(c["r"][:, o : o + H],
                                                     bank[:, :])

            def ioust(c):  # Pool: iou = ov * r (fp16 * fp32 -> fp16)
                w = c["w"]
                nc.gpsimd.tensor_tensor(c["iou"][:, :w], c["ov"][:, :w],
                                        c["r"][:, :w], Op.mult)

            def iou_psum(c):  # PE: per-seg partition-sums into PSUM banks
                t, w = c["t"], c["w"]
                nc.tensor.matmul(psA1[:, :], ones[:, :], c["iou"][:, :H],
                                 start=(t == 0), stop=(t == NSEG - 1))
                if w > H:
                    nc.tensor.matmul(psA2[:, :], ones[:, :], c["iou"][:, H : 2 * H],
                                     start=(t == BIG_LO), stop=(t == BIG_HI))

            def accum(c):  # ACT: loss partial rides the Ln accumulator
                t, w = c["t"], c["w"]
                nc.scalar.activation(
                    c["sc"][:, :w], c["iou"][:, :w], Act.Ln,
                    bias=eps_t[:, 0:1],
                    accum_out=accs[:, t : t + 1],
                )

            def ps_extract(bank, col, n):
                pscp = pool.tile([1, H], F32, tag="pscp", name=f"pscp_{col}")
                nc.scalar.copy(pscp[:, :n], bank[:, :n])
                nc.sync.dma_start(out=iouv_d[:, col : col + n],
                                  in_=pscp[:, :n])

            # Six-stage pipeline; per-iteration engine queue orders:
            #   ACT : ln(k-6) | rw,rh(k-3) | abs,squares(k-1)
            #   Pool: iou(k-5) | ov(k-3)
            #   PE  : iou_psum(k-6) | uemm(k-3) | drain(k) | spmm(k)
            #   DVE : mw,mh,wr,hr(k-2) | recip(k-4) | dummy,m3(k)
            # Program-order rules (shared PSUM banks, see docstring):
            #   recip(k-4) before uemm(k-3); mids(k-2) and absd(k-1)'s qS
            #   before spmm(k).
            for k in range(NSEG + 7):
                if 6 <= k <= NSEG + 5:
                    accum(C[k - 6])
                    iou_psum(C[k - 6])
                if 5 <= k <= NSEG + 4:
                    ioust(C[k - 5])
                if 4 <= k <= NSEG + 3:
                    recip(C[k - 4])
                if 3 <= k <= NSEG + 2:
                    relus(C[k - 3])
                    ovst(C[k - 3])
                    uemm(C[k - 3])
                if 2 <= k <= NSEG + 1:
                    q12st(C[k - 2])
                    mids(C[k - 2])
                if 1 <= k <= NSEG:
                    absd(C[k - 1])
                if k < NSEG:
                    front(k)
                    spmm(C[k])
                if k == NSEG + 5:  # psA2 closed at iter NSEG+4 (seg BIG_HI)
                    ps_extract(psA2, H, H)

            ps_extract(psA1, 0, H)
            nc.sync.dma_start(out=acc_d[:, :], in_=accs[:, :])

    if compile_passes:
        nc.compile()
    return nc


_NC_CACHE: list[bass.Bass] = []


def _get_nc() -> bass.Bass:
    if not _NC_CACHE:
        _NC_CACHE.append(_build())
    return _NC_CACHE[0]


def _run(inputs: dict, trace: bool = False, trace_kwargs: dict | None = None):
    outputs = np.ascontiguousarray(np.asarray(inputs["outputs"], dtype=np.float32))
    targets = np.ascontiguousarray(np.asarray(inputs["targets"], dtype=np.float32))
    assert outputs.shape == (N, 3) and targets.shape == (N, 3)

    nc = _get_nc()
    in_maps = [
        {
            "outputs": outputs[c * NS : (c + 1) * NS],
            "targets": targets[c * NS : (c + 1) * NS],
        }
        for c in range(NCORES)
    ]
    kw = {}
    if trace:
        kw["trace"] = True
        if trace_kwargs:
            kw["trace_kwargs"] = trace_kwargs
    res = run_bass_kernel_spmd(nc, in_maps, list(range(NCORES)), **kw)

    iou_sum = 0.0
    loss = 0.0
    for c in range(NCORES):
        acc = np.asarray(res.results[c]["acc"], dtype=np.float64)
        loss += acc.sum()
        iou_sum += np.asarray(res.results[c]["iouv"], dtype=np.float64).sum()
    loss = -loss
    return (np.float32(loss), np.float32(iou_sum)), res


def kernel(**inputs) -> tuple:
    (loss, iou_sum), _ = _run(inputs)
    return (loss, iou_sum)
